# revision 18
# baseline (speedup 1.0000x reference)
"""GCN (3x [spmm + linear]) Trainium2 Bass kernel for nn_GCNModel_75557064671960.

Contract: kernel(**inputs) takes FULL unsharded numpy inputs and returns the
FULL [50000, 64] float32 output.

Model: out = A(A(A x W1 + b1) W2 + b2) W3 + b3, where A is a 50000x50000
sparse matrix with 800k weighted edges (duplicate edges sum). Using matmul
associativity each layer is h_next = A @ (h @ W) + b.

Device strategy (8 NeuronCores, SPMD single NEFF):
- Nodes are dst-sharded: core c owns rows [c*6250, (c+1)*6250), padded to
  6272 (=49*128) local rows; padded global node space is 8*6272 = 50176.
- Per layer: every core redundantly computes the full dense GEMM
  y = H @ W (fp16 inputs, fp32 PSUM) writing node-major y to HBM; then runs
  gather(y[src]) via dma_gather, scales by edge values on the vector engine
  (free-dim step-0 broadcast AP), and accumulates into its local dst shard
  via dma_scatter_add (SDMA CCE add). Layer boundary: bias add + PE
  transpose + fp16 cast of the local shard, AllGather of the transposed
  shards so every core has the full H^T for the next GEMM.
- dma_gather/dma_scatter_add indices are int16, so the padded node space is
  split at row 32768 into two gather groups with chunk-local indices.

All graph preprocessing (dst-shard the edge list, build wrapped int16 index
tiles, pad to uniform per-core sizes) happens on host inside kernel().
"""

import os
import sys

import numpy as np

if "/opt/trn_rl_repo" not in sys.path:
    sys.path.insert(0, "/opt/trn_rl_repo")

N_NODES = 50000
N_CORES = 8
IN_DIM, HID_DIM, OUT_DIM = 128, 128, 64
CHUNK = 1024  # edges per call; chunk//16+1 SWDGE ring entries per direction
SPLIT = 32768  # int16 gather index limit

# Results of the last device run (for test harnesses).
LAST_EXEC_TIME_NS = None
LAST_TRACE = None

_PROGRAM_CACHE = {}


def _round_up(x, m):
    return (x + m - 1) // m * m


# ---------------------------------------------------------------------------
# Host-side preprocessing
# ---------------------------------------------------------------------------


def _wrap_idx16(idx, e_pad):
    """Edge index array -> [128, e_pad//16] int16 tile (i at [i%16, i//16],
    replicated 8x across partition groups of 16)."""
    a = np.zeros(e_pad, dtype=np.int16)
    a[: idx.size] = idx.astype(np.int16)
    w16 = a.reshape(-1, 16).T  # [16, e_pad//16]
    return np.ascontiguousarray(np.tile(w16, (8, 1)))


def _wrap_val(val, e_pad):
    """Edge value array -> [128, e_pad//128] f32 (gather-output layout:
    value of edge i at [i%128, i//128])."""
    a = np.zeros(e_pad, dtype=np.float32)
    a[: val.size] = val
    return np.ascontiguousarray(a.reshape(-1, 128).T)


def _preprocess(x, adj_indices, adj_values):
    """Build per-core input arrays.

    dma_scatter_add's CCE read-modify-write races when one call carries
    duplicate dst indices (HW-verified), so edges are scheduled into
    "rounds": the j-th edge of each dst node goes to round j, making dst
    indices unique within every scatter call. Rounds are laid out in a flat
    slot space [r0.A | r0.B | r1.A | ...] (A = src < SPLIT, B = rest, for
    int16 gather indices), each segment padded to a multiple of 128 with
    (src=0, dst=n_loc [trash row], val=0). Segment sizes are maxed over
    cores so the SPMD program is shape-uniform. Returns a static chunk
    schedule of (slot_start, n_slots, group) triples.
    """
    n_loc_raw = N_NODES // N_CORES  # 6250
    n_loc = _round_up(n_loc_raw, 128)  # 6272
    n_pad = n_loc * N_CORES  # 50176

    dst = np.asarray(adj_indices[0], dtype=np.int64)
    src = np.asarray(adj_indices[1], dtype=np.int64)
    val = np.asarray(adj_values, dtype=np.float32)

    core = dst // n_loc_raw
    loc_dst = (dst - core * n_loc_raw).astype(np.int64)
    gsrc = ((src // n_loc_raw) * n_loc + (src % n_loc_raw)).astype(np.int64)

    percore = []
    for c in range(N_CORES):
        m = core == c
        gs, ld, vv = gsrc[m], loc_dst[m], val[m]
        order = np.argsort(ld, kind="stable")
        gs, ld, vv = gs[order], ld[order], vv[order]
        # occurrence rank of each edge within its dst
        if ld.size:
            newd = np.r_[True, ld[1:] != ld[:-1]]
            first = np.maximum.accumulate(np.where(newd, np.arange(ld.size), 0))
            rank = np.arange(ld.size) - first
        else:
            rank = np.zeros(0, dtype=np.int64)
        grp = (gs >= SPLIT).astype(np.int64)
        percore.append((gs, ld, vv, rank, grp))

    n_rounds = max(int(pc[3].max(initial=-1)) + 1 for pc in percore)
    # per (round, group) max count over cores -> padded segment sizes
    seg = np.zeros((n_rounds, 2), dtype=np.int64)
    for gs, ld, vv, rank, grp in percore:
        for g in (0, 1):
            cnt = np.bincount(rank[grp == g], minlength=n_rounds)
            seg[:, g] = np.maximum(seg[:, g], cnt)
    seg_pad = ((seg + 127) // 128) * 128

    offs = np.zeros((n_rounds, 2), dtype=np.int64)
    total = 0
    for r in range(n_rounds):
        for g in (0, 1):
            offs[r, g] = total
            total += seg_pad[r, g]
    total = int(total)

    # chunk schedule: (slot_start, n_slots, group, round)
    schedule = []
    for r in range(n_rounds):
        for g in (0, 1):
            s, n = int(offs[r, g]), int(seg_pad[r, g])
            while n > 0:
                take = min(n, CHUNK)
                schedule.append((s, take, g, r))
                s += take
                n -= take

    per_core = []
    for gs, ld, vv, rank, grp in percore:
        src_slots = np.zeros(total, dtype=np.int64)
        dst_slots = np.full(total, n_loc, dtype=np.int64)  # trash row
        val_slots = np.zeros(total, dtype=np.float32)
        # position within (round, group) segment
        so = np.lexsort((ld, rank, grp))
        gs2, ld2, vv2, rank2, grp2 = gs[so], ld[so], vv[so], rank[so], grp[so]
        if gs2.size:
            key = grp2 * n_rounds + rank2
            newseg = np.r_[True, key[1:] != key[:-1]]
            segfirst = np.maximum.accumulate(np.where(newseg, np.arange(key.size), 0))
            pos = np.arange(key.size) - segfirst
            slots = offs[rank2, grp2] + pos
            src_slots[slots] = gs2 - grp2 * SPLIT
            dst_slots[slots] = ld2
            val_slots[slots] = vv2
        per_core.append(
            {
                "srcS": _wrap_idx16(src_slots, total),
                "dstS": _wrap_idx16(dst_slots, total),
                "valS": _wrap_val(val_slots, total),
            }
        )

    # x -> padded, per-core-block transposed fp16: [C, 128, n_loc]
    xpad = np.zeros((N_CORES, n_loc, IN_DIM), dtype=np.float16)
    xr = np.asarray(x, dtype=np.float16).reshape(N_CORES, n_loc_raw, IN_DIM)
    xpad[:, :n_loc_raw] = xr
    shared = {"xT": np.ascontiguousarray(xpad.transpose(0, 2, 1))}
    return shared, per_core, (n_loc_raw, n_loc, n_pad, total, tuple(schedule))


# ---------------------------------------------------------------------------
# Bass program
# ---------------------------------------------------------------------------


def _build_program(sizes, dims):
    """Build the SPMD Bass/Tile program. dims = [(128,128),(128,128),(128,64)]."""
    from concourse import bacc, bass, mybir, tile

    n_loc_raw, n_loc, n_pad, n_slots, schedule = sizes
    nt = n_loc // 128  # node tiles per core
    f32 = mybir.dt.float32
    f16 = mybir.dt.float16
    i16 = mybir.dt.int16

    nc = bacc.Bacc(
        "TRN2", target_bir_lowering=False, debug=False, num_devices=N_CORES
    )

    # ---- I/O -------------------------------------------------------------
    xT = nc.dram_tensor("xT", [N_CORES, 128, n_loc], f16, kind="ExternalInput")
    Ws, Bs_ = [], []
    for li, (din, dout) in enumerate(dims):
        Ws.append(
            nc.dram_tensor(f"W{li}", [din, dout], f16, kind="ExternalInput")
        )
        Bs_.append(
            nc.dram_tensor(f"b{li}", [128, dout], f32, kind="ExternalInput")
        )
    ident = nc.dram_tensor("ident", [128, 128], f32, kind="ExternalInput")
    srcS = nc.dram_tensor("srcS", [128, n_slots // 16], i16, kind="ExternalInput")
    dstS = nc.dram_tensor("dstS", [128, n_slots // 16], i16, kind="ExternalInput")
    valS = nc.dram_tensor("valS", [128, n_slots // 128], f32, kind="ExternalInput")
    out_ext = nc.dram_tensor("out", [n_loc, dims[-1][1]], f32, kind="ExternalOutput")

    # ---- internal DRAM ---------------------------------------------------
    ys = [
        nc.dram_tensor(f"y{li}", [n_pad, dout], f32, kind="Internal")
        for li, (_, dout) in enumerate(dims)
    ]
    # two alternating accumulators (+trash row block) so consecutive
    # scatter rounds don't serialize on one tensor
    haccs = [
        [
            nc.dram_tensor(f"hacc{li}_{p}", [n_loc + 128, dout], f32, kind="Internal")
            for p in range(2)
        ]
        for li, (_, dout) in enumerate(dims)
    ]
    cc_ins = [
        nc.dram_tensor(f"ccin{li}", [128, n_loc], f16, kind="Internal")
        for li in range(len(dims) - 1)
    ]
    cc_outs = [
        nc.dram_tensor(
            f"ccout{li}", [N_CORES, 128, n_loc], f16, kind="Internal",
            addr_space="Shared",
        )
        for li in range(len(dims) - 1)
    ]

    grp_rows = [(0, min(n_pad, SPLIT))]
    if n_pad > SPLIT:
        grp_rows.append((SPLIT, n_pad - SPLIT))

    with tile.TileContext(nc) as tc:
        with (
            tc.tile_pool(name="const", bufs=1) as constp,
            tc.tile_pool(name="idx", bufs=1) as idxp,
            tc.tile_pool(name="ht", bufs=2) as htp,
            tc.tile_pool(name="ysb", bufs=2) as ysbp,
            tc.tile_pool(name="msg", bufs=2) as msgp,
            tc.tile_pool(name="bnd", bufs=3) as bndp,
            tc.tile_pool(name="stage", bufs=1) as stagep,
            tc.tile_pool(name="gps", bufs=4, space="PSUM") as gpsp,
            tc.tile_pool(name="tps", bufs=2, space="PSUM") as tpsp,
        ):
            # ---- load constants to SBUF ---------------------------------
            w_sb, b_sb = [], []
            for li, (din, dout) in enumerate(dims):
                w = constp.tile([din, dout], f16, tag=f"w{li}")
                nc.sync.dma_start(w[:], Ws[li][:])
                w_sb.append(w)
                b = constp.tile([128, dout], f32, tag=f"b{li}")
                nc.sync.dma_start(b[:], Bs_[li][:])
                b_sb.append(b)
            id_sb = constp.tile([128, 128], f32, tag="ident")
            nc.sync.dma_start(id_sb[:], ident[:])
            zero_sb = constp.tile([128, 1568], f32, tag="zero")
            nc.gpsimd.memset(zero_sb[:], 0.0)

            s_sb = idxp.tile([128, n_slots // 16], i16, tag="srcS")
            d_sb = idxp.tile([128, n_slots // 16], i16, tag="dstS")
            v_sb = idxp.tile([128, n_slots // 128], f32, tag="valS")
            nc.sync.dma_start(s_sb[:], srcS[:])
            nc.sync.dma_start(d_sb[:], dstS[:])
            nc.sync.dma_start(v_sb[:], valS[:])

            n_layers = len(dims)
            max_layers = int(os.environ.get("GCN_MAX_LAYERS", "99"))
            skip_spmm = bool(int(os.environ.get("GCN_SKIP_SPMM", "0")))
            skip_bnd = bool(int(os.environ.get("GCN_SKIP_BOUNDARY", "0")))
            for li, (din, dout) in enumerate(dims):
                if li >= max_layers:
                    break
                y_t, hacc_t = ys[li], haccs[li]

                # ---- dense GEMM: y = H @ W (full, redundant per core) ----
                for cb in range(N_CORES):
                    ht = htp.tile([128, n_loc], f16, tag="ht")
                    if li == 0:
                        nc.sync.dma_start(ht[:], xT[cb])
                    else:
                        nc.sync.dma_start(ht[:], cc_outs[li - 1][cb])
                    ysb = ysbp.tile([128, nt * dout], f32, tag="ysb")
                    for j in range(nt):
                        ps = gpsp.tile([128, dout], f32, tag="gemm_ps")
                        nc.tensor.matmul(
                            ps[:],
                            ht[:, j * 128 : (j + 1) * 128],
                            w_sb[li][:],
                            start=True,
                            stop=True,
                        )
                        nc.scalar.copy(ysb[:, j * dout : (j + 1) * dout], ps[:])
                    # rows n = cb*n_loc + j*128 + p
                    ydst = y_t[cb * n_loc : (cb + 1) * n_loc].rearrange(
                        "(j p) d -> p j d", p=128
                    )
                    nc.sync.dma_start(ydst, ysb[:].rearrange("p (j d) -> p j d", d=dout))

                # ---- zero-init the two local accumulators ----------------
                for p in range(2):
                    hv = hacc_t[p][0:n_loc].rearrange("(j p) d -> p j d", p=128)
                    zstep = max(1, 1568 // dout)
                    for q in range(0, nt, zstep):
                        qn = min(zstep, nt - q)
                        nc.sync.dma_start(
                            hv[:, q : q + qn, :],
                            zero_sb[:, : qn * dout].rearrange(
                                "p (j d) -> p j d", d=dout
                            ),
                        )

                # ---- spmm: gather + scale + scatter-add ------------------
                for s0, n_ch, g, rnd in (() if skip_spmm else schedule):
                    lo, rows = grp_rows[g]
                    msg = msgp.tile([128, CHUNK // 128, dout], f32, tag="msg")
                    mv = msg[:, : n_ch // 128, :]
                    nc.gpsimd.dma_gather(
                        mv,
                        ys[li][lo : lo + rows],
                        s_sb[:, s0 // 16 : (s0 + n_ch) // 16],
                        n_ch,
                        n_ch,
                        dout,
                    )
                    vslice = v_sb[:, s0 // 128 : (s0 + n_ch) // 128]
                    vb = bass.AP(vslice.tensor, vslice.offset, vslice.ap + [[0, dout]])
                    nc.vector.tensor_mul(mv, mv, vb)
                    nc.gpsimd.dma_scatter_add(
                        hacc_t[rnd % 2][:],
                        mv,
                        d_sb[:, s0 // 16 : (s0 + n_ch) // 16],
                        n_ch,
                        n_ch,
                        dout,
                    )

                # ---- layer boundary --------------------------------------
                if skip_bnd:
                    continue
                if li < n_layers - 1:
                    # sum both accumulators + bias + fp16 cast + transpose
                    hT = stagep.tile([128, n_loc], f16, tag="hT")
                    for j in range(nt):
                        t0 = bndp.tile([128, dout], f32, tag="bnd_t0")
                        t1 = bndp.tile([128, dout], f32, tag="bnd_t1")
                        nc.sync.dma_start(t0[:], hacc_t[0][j * 128 : (j + 1) * 128])
                        nc.sync.dma_start(t1[:], hacc_t[1][j * 128 : (j + 1) * 128])
                        nc.vector.tensor_add(t0[:], t0[:], t1[:])
                        nc.vector.tensor_add(t0[:], t0[:], b_sb[li][:])
                        tp = tpsp.tile([128, 128], f32, tag="tr_ps")
                        nc.tensor.transpose(tp[:], t0[:], id_sb[:])
                        nc.scalar.copy(hT[:, j * 128 : (j + 1) * 128], tp[:])
                    nc.sync.dma_start(cc_ins[li][:], hT[:])
                    nc.gpsimd.collective_compute(
                        "AllGather",
                        mybir.AluOpType.bypass,
                        replica_groups=[list(range(N_CORES))],
                        ins=[cc_ins[li][:]],
                        outs=[cc_outs[li][:]],
                    )
                else:
                    # final: sum accumulators + bias -> ExternalOutput
                    osb = stagep.tile([128, nt * dout], f32, tag="osb")
                    for j in range(nt):
                        t0 = bndp.tile([128, dout], f32, tag="bnd_t0")
                        t1 = bndp.tile([128, dout], f32, tag="bnd_t1")
                        nc.sync.dma_start(t0[:], hacc_t[0][j * 128 : (j + 1) * 128])
                        nc.sync.dma_start(t1[:], hacc_t[1][j * 128 : (j + 1) * 128])
                        nc.vector.tensor_add(t0[:], t0[:], t1[:])
                        nc.vector.tensor_add(
                            osb[:, j * dout : (j + 1) * dout], t0[:], b_sb[li][:]
                        )
                    nc.sync.dma_start(
                        out_ext[:].rearrange("(j p) d -> p j d", p=128),
                        osb[:].rearrange("p (j d) -> p j d", d=dout),
                    )

            if os.environ.get("GCN_DEBUG_TAPS"):
                dbg_y0 = nc.dram_tensor(
                    "dbg_y0", list(ys[0].shape), f32, kind="ExternalOutput"
                )
                nc.sync.dma_start(dbg_y0[:], ys[0][:])
                for p in range(2):
                    dbg_h = nc.dram_tensor(
                        f"dbg_hacc0_{p}", list(haccs[0][p].shape), f32,
                        kind="ExternalOutput",
                    )
                    nc.sync.dma_start(dbg_h[:], haccs[0][p][:])
                dbg_cc0 = nc.dram_tensor(
                    "dbg_cc0", list(cc_outs[0].shape), f16, kind="ExternalOutput"
                )
                nc.sync.dma_start(dbg_cc0[:], cc_outs[0][:])

    nc.compile()
    return nc


# ---------------------------------------------------------------------------
# Entry point
# ---------------------------------------------------------------------------


def kernel(x, adj_indices, adj_values, W1, b1, W2, b2, W3, b3):
    global LAST_EXEC_TIME_NS, LAST_TRACE
    from concourse import bass_utils

    shared, per_core, sizes = _preprocess(x, adj_indices, adj_values)
    n_loc_raw, n_loc, n_pad, n_slots, schedule = sizes
    dims = [(IN_DIM, HID_DIM), (HID_DIM, HID_DIM), (HID_DIM, OUT_DIM)]

    key = (sizes, tuple(dims))
    if key not in _PROGRAM_CACHE:
        _PROGRAM_CACHE[key] = _build_program(sizes, dims)
    nc = _PROGRAM_CACHE[key]

    wmats = [np.asarray(w, dtype=np.float16) for w in (W1, W2, W3)]
    bias = [
        np.ascontiguousarray(
            np.broadcast_to(np.asarray(b, dtype=np.float32), (128, b.shape[-1]))
        )
        for b in (b1, b2, b3)
    ]
    ident = np.eye(128, dtype=np.float32)

    in_maps = []
    for c in range(N_CORES):
        m = {
            "xT": shared["xT"],
            "ident": ident,
            **{f"W{i}": wmats[i] for i in range(3)},
            **{f"b{i}": bias[i] for i in range(3)},
            "srcS": per_core[c]["srcS"],
            "dstS": per_core[c]["dstS"],
            "valS": per_core[c]["valS"],
        }
        in_maps.append(m)

    trace = os.environ.get("GCN_TRACE", "") not in ("", "0")
    if trace:
        _register_axon_profile_hook()

    res = bass_utils.run_bass_kernel_spmd(
        nc, in_maps, core_ids=list(range(N_CORES)), trace=trace
    )
    LAST_EXEC_TIME_NS = res.exec_time_ns
    LAST_TRACE = res.instructions_and_trace

    out = np.concatenate(
        [res.results[c]["out"][:n_loc_raw] for c in range(N_CORES)], axis=0
    )
    return np.ascontiguousarray(out.astype(np.float32))


def _register_axon_profile_hook():
    """The agent image's antenv lacks axon_hooks; inject it and register the
    ctypes NTFF profiling hook so run_bass_kernel_spmd(trace=True) works."""
    import types

    try:
        from antenv.axon_hooks import get_axon_ntff_profile_hook  # noqa: F401

        return
    except ImportError:
        pass
    import antenv

    mod = types.ModuleType("antenv.axon_hooks")
    _hook = [None]
    mod.set_axon_ntff_profile_hook = lambda h: _hook.__setitem__(0, h)
    mod.get_axon_ntff_profile_hook = lambda: _hook[0]
    sys.modules["antenv.axon_hooks"] = mod
    antenv.axon_hooks = mod
    try:
        from trn_agent_boot.trn_boot import _ntff_profile_via_ctypes

        mod.set_axon_ntff_profile_hook(
            _ntff_profile_via_ctypes("/opt/axon/libaxon_pjrt.so")
        )
    except Exception:
        pass
